# revision 3
# baseline (speedup 1.0000x reference)
"""Trainium2 Bass kernel for nn_EventWarping (contrast-maximization event
warping loss).

Strategy (data-parallel over batch, one NeuronCore per batch element):
  The core op is a bilinear scatter-add of N=262144 warped events into a
  256x256 image (4 images per warp: pos/neg polarity x {weight, weight*ts}).
  A per-item scatter is infeasible on TRN2 engines, so we use the
  TensorEngine outer-product histogram: for each chunk of 128 events build
    lhsT_pos[e, y] = wy-tent(y) * (p>0)   [128 x 256, bf16]
    lhsT_neg[e, y] = wy-tent(y) * (p<0)
    rhs[e, 0:256]  = wx-tent(x)           [128 x 512, bf16]
    rhs[e, 256:512]= wx-tent(x) * ts_w
  and accumulate  image_half += lhsT[:, half]^T @ rhs  into PSUM (f32) with
  4 matmuls per warp per chunk (2 warps -> all 8 PSUM banks, one pass).
  The bilinear 4-corner scatter falls out of the rank-1 product of the two
  2-tap tents; out-of-bounds corners drop out of the one-hot construction
  automatically (matching the reference's masking).
  Epilogue computes sum((num/(den+eps))^2)/mt^2/nonzero_px per warp plus the
  Charbonnier flow-smoothness term, all on-device; host sums the 8 per-core
  partial losses.
"""

import sys

if "/opt/trn_rl_repo" not in sys.path:
    sys.path.insert(0, "/opt/trn_rl_repo")

from contextlib import ExitStack

import ml_dtypes
import numpy as np

import concourse.bacc as bacc
import concourse.bass as bass
import concourse.mybir as mybir
from concourse.tile import TileContext

F32 = mybir.dt.float32
BF16 = mybir.dt.bfloat16
AL = mybir.AluOpType
ACTF = mybir.ActivationFunctionType

P = 128
RES = 256
NPIX = RES * RES
EPS = 1e-9
FLOW_TEMP_REG = 1e-3


def _emit(tc, ev, iotas, vecb, loss_out, N, mt, hw_loop=True):
    nc = tc.nc
    C = N // P
    stk = ExitStack()

    const_pool = stk.enter_context(tc.tile_pool(name="const", bufs=1))
    iota = const_pool.tile([P, 256], BF16)
    iotam = const_pool.tile([P, 256], BF16)
    nc.sync.dma_start(iota, iotas[:, 0:256])
    nc.sync.dma_start(iotam, iotas[:, 256:512])
    ones = const_pool.tile([P, 1], F32)
    nc.gpsimd.memset(ones, 1.0)
    zk = const_pool.tile([1, 640], BF16)
    nc.gpsimd.memset(zk, 0.0)
    vtile = const_pool.tile([1, 32], F32)
    nc.sync.dma_start(vtile, vecb)

    raw_pool = stk.enter_context(tc.tile_pool(name="raw", bufs=1))

    def load_field(f):
        t = raw_pool.tile([P, C], F32, tag=f"raw{f}", name=f"raw{f}")
        nc.sync.dma_start(t, ev[f : f + 1, :].rearrange("o (p c) -> (o p) c", p=P))
        return t

    ts_t, y_t, x_t, p_t, fy_t, fx_t = [load_field(f) for f in range(6)]

    fld_pool = stk.enter_context(tc.tile_pool(name="fld", bufs=1))
    d0 = fld_pool.tile([P, C], F32)
    # d0 = mt - ts  (also the ts weight for warp 1)
    nc.vector.tensor_scalar(d0, ts_t, -1.0, float(mt), AL.mult, AL.add)

    scr = fld_pool.tile([P, C], F32, tag="scr")
    scri = fld_pool.tile([P, C], mybir.dt.int32, tag="scri")

    warps = []
    for w in (0, 1):
        wt = {}
        for name, coord, flow in (("y", y_t, fy_t), ("x", x_t, fx_t)):
            pos = fld_pool.tile([P, C], F32, tag=f"w{w}{name}0", name=f"w{w}{name}0")
            if w == 0:
                nc.vector.tensor_tensor(out=scr, in0=d0, in1=flow, op=AL.mult)
                nc.vector.tensor_tensor(out=pos, in0=coord, in1=scr, op=AL.add)
            else:
                nc.vector.tensor_tensor(out=scr, in0=ts_t, in1=flow, op=AL.mult)
                nc.vector.tensor_tensor(out=pos, in0=coord, in1=scr, op=AL.subtract)
            # floor via int-cast + correction (DVE has no mod/floor op):
            #   r = f32(int32(wpos)); floor = r - (r > wpos); frac = wpos - floor
            flr = fld_pool.tile([P, C], F32, tag=f"w{w}{name}flr", name=f"w{w}{name}flr")
            nc.vector.tensor_copy(out=scri, in_=pos)
            nc.vector.tensor_copy(out=flr, in_=scri)
            nc.vector.tensor_tensor(out=scr, in0=flr, in1=pos, op=AL.is_gt)
            nc.vector.tensor_tensor(out=flr, in0=flr, in1=scr, op=AL.subtract)
            # pos becomes the fractional part in place
            nc.vector.tensor_tensor(out=pos, in0=pos, in1=flr, op=AL.subtract)
            f0 = fld_pool.tile([P, C], F32, tag=f"w{w}{name}f0", name=f"w{w}{name}f0")
            nc.vector.tensor_scalar(f0, pos, -1.0, 1.0, AL.mult, AL.add)
            wt[f"{name}0"] = flr
            wt[f"{name}f1"] = pos
            wt[f"{name}f0"] = f0
        wt["t"] = ts_t if w == 0 else d0
        warps.append(wt)

    pos01 = fld_pool.tile([P, C], F32)
    nc.vector.tensor_scalar(pos01, p_t, 0.5, 0.5, AL.mult, AL.add)
    neg01 = fld_pool.tile([P, C], F32)
    nc.vector.tensor_scalar(neg01, pos01, -1.0, 1.0, AL.mult, AL.add)

    psum_pool = tc.tile_pool(name="psum", bufs=1, space="PSUM")
    psum = psum_pool.__enter__()
    # U = pos-polarity images, S = neg-polarity images; [w][half]
    U = [
        [psum.tile([P, 512], F32, tag=f"U{w}{h}", name=f"U{w}{h}") for h in (0, 1)]
        for w in (0, 1)
    ]
    S = [
        [psum.tile([P, 512], F32, tag=f"S{w}{h}", name=f"S{w}{h}") for h in (0, 1)]
        for w in (0, 1)
    ]

    zl = zk[0:1, 0:128]
    zr = zk[0:1, 128:640]
    for w in (0, 1):
        for h in (0, 1):
            nc.tensor.matmul(out=U[w][h][:], lhsT=zl, rhs=zr, start=True, stop=False)
            nc.tensor.matmul(out=S[w][h][:], lhsT=zl, rhs=zr, start=True, stop=False)

    loop_pool = stk.enter_context(tc.tile_pool(name="loop", bufs=2))

    def chunk_body(i, base, span):
        def col(t):
            # static base offset + small register offset: the HW register
            # path for dynamic AP offsets only covers ~2KB, so the python-
            # level block loop carries the large part statically.
            return t[:, base : base + span][:, bass.ds(i, 1)]

        for w in (0, 1):
            wt = warps[w]
            y0c = col(wt["y0"])
            x0c = col(wt["x0"])
            fy0c = col(wt["yf0"])
            fy1c = col(wt["yf1"])
            fx0c = col(wt["xf0"])
            fx1c = col(wt["xf1"])
            tc_ = col(wt["t"])
            posc = col(pos01)
            negc = col(neg01)

            u0 = loop_pool.tile([P, 256], BF16, tag=f"u0{w}", name=f"u0{w}")
            nc.vector.tensor_scalar(u0, iota, y0c, fy0c, AL.is_equal, AL.mult)
            eqm = loop_pool.tile([P, 256], BF16, tag=f"eqm{w}", name=f"eqm{w}")
            nc.vector.tensor_scalar(eqm, iotam, y0c, fy1c, AL.is_equal, AL.mult)
            nc.vector.tensor_tensor(out=u0, in0=u0, in1=eqm, op=AL.add)
            u = loop_pool.tile([P, 256], BF16, tag=f"u{w}", name=f"u{w}")
            nc.vector.tensor_scalar(u, u0, posc, None, AL.mult)
            s = loop_pool.tile([P, 256], BF16, tag=f"s{w}", name=f"s{w}")
            nc.vector.tensor_scalar(s, u0, negc, None, AL.mult)

            r = loop_pool.tile([P, 512], BF16, tag=f"r{w}", name=f"r{w}")
            r0 = r[:, 0:256]
            r1 = r[:, 256:512]
            nc.vector.tensor_scalar(r0, iota, x0c, fx0c, AL.is_equal, AL.mult)
            eqm2 = loop_pool.tile([P, 256], BF16, tag=f"eqm2{w}", name=f"eqm2{w}")
            nc.vector.tensor_scalar(eqm2, iotam, x0c, fx1c, AL.is_equal, AL.mult)
            nc.vector.tensor_tensor(out=r0, in0=r0, in1=eqm2, op=AL.add)
            nc.vector.tensor_scalar(r1, r0, tc_, None, AL.mult)

            for h in (0, 1):
                nc.tensor.matmul(
                    out=U[w][h][:],
                    lhsT=u[:, h * 128 : (h + 1) * 128],
                    rhs=r[:],
                    start=False,
                    stop=False,
                )
                nc.tensor.matmul(
                    out=S[w][h][:],
                    lhsT=s[:, h * 128 : (h + 1) * 128],
                    rhs=r[:],
                    start=False,
                    stop=False,
                )

    CB = 512
    if hw_loop:
        for b in range(0, C, CB):
            span = min(CB, C - b)
            with tc.For_i(0, span) as i:
                chunk_body(i, b, span)
    else:
        for i in range(C):
            chunk_body(i, 0, C)

    for w in (0, 1):
        for h in (0, 1):
            nc.tensor.matmul(out=U[w][h][:], lhsT=zl, rhs=zr, start=False, stop=True)
            nc.tensor.matmul(out=S[w][h][:], lhsT=zl, rhs=zr, start=False, stop=True)

    epi_pool = stk.enter_context(tc.tile_pool(name="epi", bufs=1))
    rows = epi_pool.tile([P, 4], F32)
    den = epi_pool.tile([P, 256], F32, tag="den")
    num = epi_pool.tile([P, 256], F32, tag="num")
    rec = epi_pool.tile([P, 256], F32, tag="rec")
    for w in (0, 1):
        SQ = epi_pool.tile([P, 256], F32, tag=f"SQ{w}", name=f"SQ{w}")
        Z = epi_pool.tile([P, 256], F32, tag=f"Z{w}", name=f"Z{w}")
        nc.vector.memset(SQ, 0.0)
        nc.vector.memset(Z, 0.0)
        for h in (0, 1):
            Uh, Sh = U[w][h], S[w][h]
            for img in (Uh, Sh):
                nc.vector.tensor_scalar(den, img[:, 0:256], EPS, None, AL.add)
                nc.vector.reciprocal(rec, den)
                nc.vector.tensor_tensor(
                    out=num, in0=img[:, 256:512], in1=rec, op=AL.mult
                )
                nc.vector.tensor_tensor(out=num, in0=num, in1=num, op=AL.mult)
                nc.vector.tensor_tensor(out=SQ, in0=SQ, in1=num, op=AL.add)
            # nonzero-pixel count uses iwe_pos + iwe_neg
            # (only one tensor_tensor input may come from PSUM -> stage S)
            nc.vector.tensor_copy(out=rec, in_=Sh[:, 0:256])
            nc.vector.tensor_tensor(out=den, in0=Uh[:, 0:256], in1=rec, op=AL.add)
            nc.vector.tensor_scalar(den, den, 0.0, None, AL.is_equal)
            nc.vector.tensor_tensor(out=Z, in0=Z, in1=den, op=AL.add)
        nc.vector.tensor_reduce(
            out=rows[:, 2 * w : 2 * w + 1], in_=SQ, axis=mybir.AxisListType.X, op=AL.add
        )
        nc.vector.tensor_reduce(
            out=rows[:, 2 * w + 1 : 2 * w + 2],
            in_=Z,
            axis=mybir.AxisListType.X,
            op=AL.add,
        )

    psum_pool.__exit__(None, None, None)

    with tc.tile_pool(name="psum2", bufs=1, space="PSUM") as psum2:
        red = psum2.tile([1, 4], F32)
        nc.tensor.matmul(out=red[:], lhsT=ones[:], rhs=rows[:], start=True, stop=True)
        scal = epi_pool.tile([1, 4], F32)
        nc.vector.tensor_copy(out=scal, in_=red[:])

    lt = epi_pool.tile([1, 1], F32)
    nc.vector.memset(lt, 0.0)
    t1 = epi_pool.tile([1, 1], F32)
    t2 = epi_pool.tile([1, 1], F32)
    for w in (0, 1):
        # t1 = 65536 - zero_count  (the reference's +EPS is an f32 no-op here)
        nc.vector.tensor_scalar(
            t1, scal[0:1, 2 * w + 1 : 2 * w + 2], -1.0, float(NPIX), AL.mult, AL.add
        )
        nc.vector.reciprocal(t2, t1)
        nc.vector.tensor_scalar(
            t1, scal[0:1, 2 * w : 2 * w + 1], 1.0 / (mt * mt), None, AL.mult
        )
        nc.vector.scalar_tensor_tensor(lt, t1, t2, lt, AL.mult, AL.add)

    # Charbonnier temporal-smoothness on vector_list
    d24 = epi_pool.tile([1, 24], F32)
    nc.vector.tensor_tensor(
        out=d24, in0=vtile[0:1, 0:24], in1=vtile[0:1, 8:32], op=AL.subtract
    )
    epsb = epi_pool.tile([1, 1], F32)
    nc.vector.memset(epsb, EPS)
    nc.scalar.activation(d24, d24, ACTF.Square)
    nc.scalar.activation(d24, d24, ACTF.Sqrt, bias=epsb[0:1, 0:1])
    ch = epi_pool.tile([1, 1], F32)
    nc.vector.tensor_reduce(out=ch, in_=d24, axis=mybir.AxisListType.X, op=AL.add)
    nc.vector.scalar_tensor_tensor(lt, ch, FLOW_TEMP_REG / 24.0, lt, AL.mult, AL.add)

    nc.sync.dma_start(loss_out, lt[:])
    stk.close()


def _build(N, mt, hw_loop=True, num_devices=8):
    nc = bacc.Bacc(
        "TRN2", target_bir_lowering=False, debug=False, num_devices=num_devices
    )
    ev = nc.dram_tensor("ev", [6, N], F32, kind="ExternalInput")
    iotas = nc.dram_tensor("iotas", [P, 512], BF16, kind="ExternalInput")
    vecb = nc.dram_tensor("vecb", [1, 32], F32, kind="ExternalInput")
    loss = nc.dram_tensor("loss", [1, 1], F32, kind="ExternalOutput")
    with TileContext(nc) as tc:
        _emit(tc, ev.ap(), iotas.ap(), vecb.ap(), loss.ap(), N, mt, hw_loop)
    nc.compile()
    return nc


def _host_iotas():
    a = np.arange(256, dtype=np.float32)
    io = np.concatenate([a, a - 1.0])
    return np.tile(io[None, :], (P, 1)).astype(ml_dtypes.bfloat16)


def _pack_inputs(event_list, flow, vector_list):
    B = event_list.shape[0]
    iot = _host_iotas()
    maps = []
    for b in range(B):
        ev6 = np.ascontiguousarray(
            np.concatenate([event_list[b].T, flow[b].T], axis=0), dtype=np.float32
        )
        vecb = np.ascontiguousarray(vector_list[b].reshape(1, 32), dtype=np.float32)
        maps.append({"ev": ev6, "iotas": iot, "vecb": vecb})
    return maps


_NC_CACHE = {}
LAST_RESULT = None  # BassKernelResults of the most recent run (for test.py)


def kernel(event_list, flow, pol_mask, vector_list, max_ts):
    from concourse.bass_utils import run_bass_kernel_spmd

    event_list = np.asarray(event_list)
    flow = np.asarray(flow)
    vector_list = np.asarray(vector_list)
    B, N, _ = event_list.shape
    mt = float(np.asarray(max_ts))

    key = (N, mt, B)
    nc = _NC_CACHE.get(key)
    if nc is None:
        nc = _build(N, mt, hw_loop=True, num_devices=B)
        _NC_CACHE[key] = nc

    in_maps = _pack_inputs(event_list, flow, vector_list)
    res = run_bass_kernel_spmd(nc, in_maps, core_ids=list(range(B)))
    global LAST_RESULT
    LAST_RESULT = res
    vals = np.array(
        [res.results[b]["loss"][0, 0] for b in range(B)], dtype=np.float32
    )
    return np.float32(np.sum(vals, dtype=np.float32))



# revision 14
# speedup vs baseline: 3.2425x; 3.2425x over previous
"""Trainium2 Bass kernel for nn_EventWarping (contrast-maximization event
warping loss).

Strategy (data-parallel over batch, one NeuronCore per batch element):
  Bilinear scatter-add of N=262144 warped events into 256x256 images via the
  TensorEngine outer-product histogram: per chunk of 128 events,
    image[y, (x|x*ts)] += ty[e, y]^T @ rhs[e, (x|x*ts)]
  where ty is the 2-tap y-tent and rhs packs the x-tent and its ts-weighted
  copy.  Events are polarity-sorted on the host (scatter-add is permutation
  invariant), so each chunk touches only its polarity's 4 PSUM banks: this
  halves both the matmul count and the on-device masking work vs an unsorted
  kernel.  Engine split per chunk: DVE computes d = iota - wy and q = |d|-1,
  ACT applies relu(-q) to finish the tents, GpSimd local_scatter builds the
  512-wide rhs in one op from precomputed int16 indices + bf16 data, and the
  PE accumulates 4 (ldweights+matmul) pairs.  4 chunks per hardware-loop
  iteration amortize loop scaffolding.
  Epilogue computes sum((num/(den+eps))^2)/mt^2/nonzero_px per warp plus the
  Charbonnier flow-smoothness term on device; host sums the 8 per-core
  losses.
"""

import sys

if "/opt/trn_rl_repo" not in sys.path:
    sys.path.insert(0, "/opt/trn_rl_repo")

from contextlib import ExitStack

import ml_dtypes
import numpy as np

import concourse.bacc as bacc
import concourse.bass as bass
import concourse.mybir as mybir
from concourse.tile import TileContext

F32 = mybir.dt.float32
BF16 = mybir.dt.bfloat16
I16 = mybir.dt.int16
I32 = mybir.dt.int32
AL = mybir.AluOpType
ACTF = mybir.ActivationFunctionType

P = 128
RES = 256
NPIX = RES * RES
EPS = 1e-9
FLOW_TEMP_REG = 1e-3
PAD_POS = -1e4  # pad-event coordinate: far out of bounds, zero contribution

U = 4     # chunks per hw-loop iteration
CB = 256  # chunks per block (dynamic-offset register range)


def _emit(tc, ev, iotas, vecb, loss_out, C2, mt):
    """C2 = total chunks (pos segment PC + neg segment PC, PC = C2//2)."""
    nc = tc.nc
    PC = C2 // 2
    stk = ExitStack()

    const_pool = stk.enter_context(tc.tile_pool(name="const", bufs=1))
    iota = const_pool.tile([P, 256], BF16)
    nc.sync.dma_start(iota, iotas[:, 0:256])
    iota_n = const_pool.tile([P, 256], BF16)
    nc.sync.dma_start(iota_n, iotas[:, 256:512])
    ones = const_pool.tile([P, 1], F32)
    nc.gpsimd.memset(ones, 1.0)
    zk = const_pool.tile([1, 640], BF16)
    nc.gpsimd.memset(zk, 0.0)
    vtile = const_pool.tile([1, 32], F32)
    nc.sync.dma_start(vtile, vecb)

    # persistent per-event tensors used inside the chunk loop
    pers_pool = stk.enter_context(tc.tile_pool(name="pers", bufs=1))
    # wy1m = 1 - wy ; wyp1 = wy + 1 (ptr-scalar sources for the y-tent)
    wy1m = [pers_pool.tile([P, C2], F32, tag=f"wy1m{w}", name=f"wy1m{w}")
            for w in (0, 1)]
    wyp1 = [pers_pool.tile([P, C2], F32, tag=f"wyp1{w}", name=f"wyp1{w}")
            for w in (0, 1)]
    idxw = [pers_pool.tile([P, 4 * C2], I16, tag=f"idx{w}", name=f"idx{w}")
            for w in (0, 1)]
    datw = [pers_pool.tile([P, 4 * C2], BF16, tag=f"dat{w}", name=f"dat{w}")
            for w in (0, 1)]

    # ---- prep: wide [P, C2] field math on DVE ----
    with tc.tile_pool(name="raw", bufs=1) as raw_pool:
        def load_field(f):
            t = raw_pool.tile([P, C2], F32, tag=f"raw{f}", name=f"raw{f}")
            nc.sync.dma_start(t, ev[f : f + 1, :].rearrange("o (p c) -> (o p) c", p=P))
            return t

        ts_t, y_t, x_t, fy_t, fx_t = [load_field(f) for f in range(5)]

        d0 = raw_pool.tile([P, C2], F32, tag="d0")
        nc.vector.tensor_scalar(d0, ts_t, -1.0, float(mt), AL.mult, AL.add)
        scr = raw_pool.tile([P, C2], F32, tag="scr")
        scr2 = raw_pool.tile([P, C2], F32, tag="scr2")
        scri = raw_pool.tile([P, C2], I32, tag="scri")

        # warped positions; wys is scratch, wx0/wx1 persist through x-prep
        wys = raw_pool.tile([P, C2], F32, tag="wys")
        wx = [raw_pool.tile([P, C2], F32, tag=f"wx{w}", name=f"wx{w}")
              for w in (0, 1)]
        for w, tw in ((0, d0), (1, ts_t)):
            op = AL.add if w == 0 else AL.subtract
            nc.vector.tensor_tensor(out=scr, in0=tw, in1=fy_t, op=AL.mult)
            nc.vector.tensor_tensor(out=wys, in0=y_t, in1=scr, op=op)
            nc.vector.tensor_scalar(wy1m[w], wys, -1.0, 1.0, AL.mult, AL.add)
            nc.vector.tensor_scalar(wyp1[w], wys, 1.0, None, AL.add)
            nc.vector.tensor_tensor(out=scr, in0=tw, in1=fx_t, op=AL.mult)
            nc.vector.tensor_tensor(out=wx[w], in0=x_t, in1=scr, op=op)

        # x-side scatter indices and data per warp
        # (y_t/fy_t/x_t/fx_t are dead now -- reuse their buffers)
        flr = y_t
        f1 = fy_t
        m0 = fx_t
        m1 = x_t
        for w, tw in ((0, ts_t), (1, d0)):
            xc = wx[w]
            nc.vector.tensor_scalar(xc, xc, 300.0, -4.0, AL.min, AL.max)
            # floor
            nc.vector.tensor_copy(out=scri, in_=xc)
            nc.vector.tensor_copy(out=flr, in_=scri)
            nc.vector.tensor_tensor(out=scr, in0=flr, in1=xc, op=AL.is_gt)
            nc.vector.tensor_tensor(out=flr, in0=flr, in1=scr, op=AL.subtract)
            nc.vector.tensor_tensor(out=f1, in0=xc, in1=flr, op=AL.subtract)

            idx4 = idxw[w][:, 0 : 4 * C2].rearrange("p (c f) -> p c f", f=4)
            dat4 = datw[w][:, 0 : 4 * C2].rearrange("p (c f) -> p c f", f=4)

            # tap validity masks
            nc.vector.tensor_scalar(m0, flr, 255.5, None, AL.is_le)
            nc.vector.tensor_scalar(m1, flr, 254.5, None, AL.is_le)
            # i0 = (flr+1)*m0 - 1 ; i1 = (flr+2)*m1 - 1
            nc.vector.tensor_scalar(scr, flr, 1.0, None, AL.add)
            nc.vector.tensor_tensor(out=scr, in0=scr, in1=m0, op=AL.mult)
            nc.vector.tensor_scalar(scr, scr, -1.0, None, AL.add)
            nc.vector.tensor_copy(out=idx4[:, :, 0], in_=scr)
            nc.vector.tensor_scalar(scr, flr, 2.0, None, AL.add)
            nc.vector.tensor_tensor(out=scr, in0=scr, in1=m1, op=AL.mult)
            nc.vector.tensor_scalar(scr, scr, -1.0, None, AL.add)
            nc.vector.tensor_copy(out=idx4[:, :, 1], in_=scr)
            # upper-half taps need flr >= 0 (resp. >= -1) on top of m0/m1
            nc.vector.tensor_scalar(scr2, flr, -0.5, None, AL.is_ge)
            nc.vector.tensor_tensor(out=scr2, in0=scr2, in1=m0, op=AL.mult)
            nc.vector.tensor_scalar(scr, flr, 257.0, None, AL.add)
            nc.vector.tensor_tensor(out=scr, in0=scr, in1=scr2, op=AL.mult)
            nc.vector.tensor_scalar(scr, scr, -1.0, None, AL.add)
            nc.vector.tensor_copy(out=idx4[:, :, 2], in_=scr)
            nc.vector.tensor_scalar(scr2, flr, -1.5, None, AL.is_ge)
            nc.vector.tensor_tensor(out=scr2, in0=scr2, in1=m1, op=AL.mult)
            nc.vector.tensor_scalar(scr, flr, 258.0, None, AL.add)
            nc.vector.tensor_tensor(out=scr, in0=scr, in1=scr2, op=AL.mult)
            nc.vector.tensor_scalar(scr, scr, -1.0, None, AL.add)
            nc.vector.tensor_copy(out=idx4[:, :, 3], in_=scr)
            # data: [f0, f1, f0*t, f1*t]
            nc.vector.tensor_scalar(scr, f1, -1.0, 1.0, AL.mult, AL.add)  # f0
            nc.vector.tensor_copy(out=dat4[:, :, 0], in_=scr)
            nc.vector.tensor_copy(out=dat4[:, :, 1], in_=f1)
            nc.vector.tensor_tensor(out=scr, in0=scr, in1=tw, op=AL.mult)
            nc.vector.tensor_copy(out=dat4[:, :, 2], in_=scr)
            nc.vector.tensor_tensor(out=scr, in0=f1, in1=tw, op=AL.mult)
            nc.vector.tensor_copy(out=dat4[:, :, 3], in_=scr)

    psum_pool = tc.tile_pool(name="psum", bufs=1, space="PSUM")
    psum = psum_pool.__enter__()
    # IMG[pol][w][h]: den in cols 0:256, num in cols 256:512
    IMG = [
        [
            [psum.tile([P, 512], F32, tag=f"I{p_}{w}{h}", name=f"I{p_}{w}{h}")
             for h in (0, 1)]
            for w in (0, 1)
        ]
        for p_ in (0, 1)
    ]

    zl = zk[0:1, 0:128]
    zr = zk[0:1, 128:640]
    for p_ in (0, 1):
        for w in (0, 1):
            for h in (0, 1):
                nc.tensor.matmul(out=IMG[p_][w][h][:], lhsT=zl, rhs=zr,
                                 start=True, stop=False)

    loop_pool = stk.enter_context(tc.tile_pool(name="loop", bufs=2))

    def chunk_body(pol, base, span, i, u):
        for w in (0, 1):
            wy1mc = wy1m[w][:, base + u : base + span : U][:, bass.ds(i, 1)]
            wyp1c = wyp1[w][:, base + u : base + span : U][:, bass.ds(i, 1)]
            idx4 = idxw[w][:, 4 * base : 4 * (base + span)].rearrange(
                "p (c f) -> p c f", f=4
            )
            idxc = idx4[:, u : span : U, :][:, bass.ds(i, 1), :]
            dat4 = datw[w][:, 4 * base : 4 * (base + span)].rearrange(
                "p (c f) -> p c f", f=4
            )
            datc = dat4[:, u : span : U, :][:, bass.ds(i, 1), :]

            # y-tent: u = 1+d, tpre = min(1-d, 1+d) = 1-|d|, ty = relu(tpre)
            ut = loop_pool.tile([P, 256], BF16, tag=f"u{w}{u}", name=f"u{w}{u}")
            nc.vector.tensor_scalar(ut, iota, wy1mc, None, AL.add)
            tp = loop_pool.tile([P, 256], BF16, tag=f"p{w}{u}", name=f"p{w}{u}")
            nc.vector.scalar_tensor_tensor(tp, iota_n, wyp1c, ut, AL.add, AL.min)
            ty = loop_pool.tile([P, 256], BF16, tag=f"t{w}{u}", name=f"t{w}{u}")
            nc.scalar.activation(ty, tp, ACTF.Relu)

            rhs = loop_pool.tile([P, 512], BF16, tag=f"r{w}{u}", name=f"r{w}{u}")
            nc.gpsimd.local_scatter(rhs[:], datc, idxc, channels=P,
                                    num_elems=512, num_idxs=4)

            for h in (0, 1):
                nc.tensor.matmul(
                    out=IMG[pol][w][h][:],
                    lhsT=ty[:, h * 128 : (h + 1) * 128],
                    rhs=rhs[:],
                    start=False,
                    stop=False,
                )

    for pol in (0, 1):
        seg0 = pol * PC
        for b in range(0, PC, CB):
            span = min(CB, PC - b)
            with tc.For_i(0, span // U) as i:
                for u in range(U):
                    chunk_body(pol, seg0 + b, span, i, u)

    for p_ in (0, 1):
        for w in (0, 1):
            for h in (0, 1):
                nc.tensor.matmul(out=IMG[p_][w][h][:], lhsT=zl, rhs=zr,
                                 start=False, stop=True)

    # ---- epilogue ----
    epi_pool = stk.enter_context(tc.tile_pool(name="epi", bufs=1))
    rows = epi_pool.tile([P, 4], F32)
    den = epi_pool.tile([P, 256], F32, tag="den")
    num = epi_pool.tile([P, 256], F32, tag="num")
    rec = epi_pool.tile([P, 256], F32, tag="rec")
    for w in (0, 1):
        SQ = epi_pool.tile([P, 256], F32, tag=f"SQ{w}", name=f"SQ{w}")
        Z = epi_pool.tile([P, 256], F32, tag=f"Z{w}", name=f"Z{w}")
        nc.vector.memset(SQ, 0.0)
        nc.vector.memset(Z, 0.0)
        for h in (0, 1):
            Ph, Nh = IMG[0][w][h], IMG[1][w][h]
            for img in (Ph, Nh):
                nc.vector.tensor_scalar(den, img[:, 0:256], EPS, None, AL.add)
                nc.vector.reciprocal(rec, den)
                nc.vector.tensor_tensor(out=num, in0=img[:, 256:512], in1=rec,
                                        op=AL.mult)
                nc.vector.tensor_tensor(out=num, in0=num, in1=num, op=AL.mult)
                nc.vector.tensor_tensor(out=SQ, in0=SQ, in1=num, op=AL.add)
            # nonzero-pixel count uses den_pos + den_neg
            # (only one tensor_tensor input may come from PSUM -> stage N)
            nc.vector.tensor_copy(out=rec, in_=Nh[:, 0:256])
            nc.vector.tensor_tensor(out=den, in0=Ph[:, 0:256], in1=rec, op=AL.add)
            nc.vector.tensor_scalar(den, den, 0.0, None, AL.is_equal)
            nc.vector.tensor_tensor(out=Z, in0=Z, in1=den, op=AL.add)
        nc.vector.tensor_reduce(
            out=rows[:, 2 * w : 2 * w + 1], in_=SQ, axis=mybir.AxisListType.X,
            op=AL.add,
        )
        nc.vector.tensor_reduce(
            out=rows[:, 2 * w + 1 : 2 * w + 2], in_=Z,
            axis=mybir.AxisListType.X, op=AL.add,
        )

    psum_pool.__exit__(None, None, None)

    with tc.tile_pool(name="psum2", bufs=1, space="PSUM") as psum2:
        red = psum2.tile([1, 4], F32)
        nc.tensor.matmul(out=red[:], lhsT=ones[:], rhs=rows[:], start=True,
                         stop=True)
        scal = epi_pool.tile([1, 4], F32)
        nc.vector.tensor_copy(out=scal, in_=red[:])

    lt = epi_pool.tile([1, 1], F32)
    nc.vector.memset(lt, 0.0)
    t1 = epi_pool.tile([1, 1], F32)
    t2 = epi_pool.tile([1, 1], F32)
    for w in (0, 1):
        # t1 = 65536 - zero_count (the reference's +EPS is an f32 no-op here)
        nc.vector.tensor_scalar(
            t1, scal[0:1, 2 * w + 1 : 2 * w + 2], -1.0, float(NPIX), AL.mult,
            AL.add,
        )
        nc.vector.reciprocal(t2, t1)
        nc.vector.tensor_scalar(
            t1, scal[0:1, 2 * w : 2 * w + 1], 1.0 / (mt * mt), None, AL.mult
        )
        nc.vector.scalar_tensor_tensor(lt, t1, t2, lt, AL.mult, AL.add)

    # Charbonnier temporal-smoothness on vector_list
    d24 = epi_pool.tile([1, 24], F32)
    nc.vector.tensor_tensor(
        out=d24, in0=vtile[0:1, 0:24], in1=vtile[0:1, 8:32], op=AL.subtract
    )
    epsb = epi_pool.tile([1, 1], F32)
    nc.vector.memset(epsb, EPS)
    nc.scalar.activation(d24, d24, ACTF.Square)
    nc.scalar.activation(d24, d24, ACTF.Sqrt, bias=epsb[0:1, 0:1])
    ch = epi_pool.tile([1, 1], F32)
    nc.vector.tensor_reduce(out=ch, in_=d24, axis=mybir.AxisListType.X, op=AL.add)
    nc.vector.scalar_tensor_tensor(lt, ch, FLOW_TEMP_REG / 24.0, lt, AL.mult,
                                   AL.add)

    nc.sync.dma_start(loss_out, lt[:])
    stk.close()


def _build(C2, mt, num_devices=8):
    nc = bacc.Bacc(
        "TRN2", target_bir_lowering=False, debug=False, num_devices=num_devices
    )
    N2 = C2 * P
    ev = nc.dram_tensor("ev", [5, N2], F32, kind="ExternalInput")
    iotas = nc.dram_tensor("iotas", [P, 512], BF16, kind="ExternalInput")
    vecb = nc.dram_tensor("vecb", [1, 32], F32, kind="ExternalInput")
    loss = nc.dram_tensor("loss", [1, 1], F32, kind="ExternalOutput")
    with TileContext(nc) as tc:
        _emit(tc, ev.ap(), iotas.ap(), vecb.ap(), loss.ap(), C2, mt)
    nc.compile()
    return nc


def _host_iotas():
    a = np.arange(256, dtype=np.float32)
    io = np.concatenate([a, -a])
    return np.tile(io[None, :], (P, 1)).astype(ml_dtypes.bfloat16)


def _pack_inputs(event_list, flow, pol_mask):
    """Polarity-partition each batch's events, pad each segment to SEGE
    events, lay out as [5, N2] with field matrices [128, C2] flattened
    row-major (chunk = column)."""
    B, N, _ = event_list.shape
    pos_masks = [pol_mask[b, :, 0] > 0.5 for b in range(B)]
    counts = [int(m.sum()) for m in pos_masks]
    maxseg = max(max(c for c in counts), max(N - c for c in counts))
    SEGE = -(-maxseg // (P * U)) * (P * U)  # multiple of 512 events
    PC = SEGE // P
    C2 = 2 * PC

    iot = _host_iotas()
    maps = []
    for b in range(B):
        m = pos_masks[b]
        fields = np.empty((5, P, C2), np.float32)
        ev5 = np.stack([
            event_list[b, :, 0], event_list[b, :, 1], event_list[b, :, 2],
            flow[b, :, 0], flow[b, :, 1],
        ])  # [5, N] (ts, y, x, fy, fx)
        for seg, sel in ((0, m), (1, ~m)):
            data = ev5[:, sel]  # [5, n]
            n = data.shape[1]
            pad = np.zeros((5, SEGE - n), np.float32)
            pad[1:3, :] = PAD_POS  # y, x out of bounds; ts=0, flow=0
            segdata = np.concatenate([data, pad], axis=1)  # [5, SEGE]
            fields[:, :, seg * PC : (seg + 1) * PC] = segdata.reshape(5, P, PC)
        ev_flat = np.ascontiguousarray(fields.reshape(5, P * C2))
        maps.append({"ev": ev_flat, "iotas": iot})
    return maps, C2


_NC_CACHE = {}
LAST_RESULT = None  # BassKernelResults of the most recent run (for test.py)


def kernel(event_list, flow, pol_mask, vector_list, max_ts):
    from concourse.bass_utils import run_bass_kernel_spmd

    event_list = np.asarray(event_list)
    flow = np.asarray(flow)
    pol_mask = np.asarray(pol_mask)
    vector_list = np.asarray(vector_list)
    B, N, _ = event_list.shape
    mt = float(np.asarray(max_ts))

    in_maps, C2 = _pack_inputs(event_list, flow, pol_mask)
    for b in range(B):
        in_maps[b]["vecb"] = np.ascontiguousarray(
            vector_list[b].reshape(1, 32), dtype=np.float32
        )

    key = (C2, mt, B)
    nc = _NC_CACHE.get(key)
    if nc is None:
        nc = _build(C2, mt, num_devices=B)
        _NC_CACHE[key] = nc

    res = run_bass_kernel_spmd(nc, in_maps, core_ids=list(range(B)))
    global LAST_RESULT
    LAST_RESULT = res
    vals = np.array(
        [res.results[b]["loss"][0, 0] for b in range(B)], dtype=np.float32
    )
    return np.float32(np.sum(vals, dtype=np.float32))


# revision 15
# speedup vs baseline: 4.2711x; 1.3172x over previous
"""Trainium2 Bass kernel for nn_EventWarping (contrast-maximization event
warping loss).

Strategy (data-parallel over batch, one NeuronCore per batch element):
  Bilinear scatter-add of N=262144 warped events into 256x256 images via the
  TensorEngine outer-product histogram: per chunk of 128 events,
    image[y, (x|x*ts)] += ty[e, y]^T @ rhs[e, (x|x*ts)]
  where ty is the 2-tap y-tent and rhs packs the x-tent and its ts-weighted
  copy.  Events are polarity-sorted on the host (scatter-add is permutation
  invariant), so each chunk touches only its polarity's 4 PSUM banks,
  halving the matmul count vs an unsorted kernel.
  All four per-chunk operand tiles (two y-tents + two 512-wide rhs) are
  built by a single GpSimd local_scatter into one [128,1536] tile from
  precomputed int16 indices and bf16 data (12 taps per event, out-of-bounds
  taps mapped to negative indices which local_scatter drops).  The PE then
  runs 4 (ldweights+matmul) pairs per chunk into 8 PSUM banks.  8 chunks per
  hardware-loop iteration amortize loop scaffolding; DVE and ACT stay out of
  the steady-state loop entirely.
  Epilogue computes sum((num/(den+eps))^2)/mt^2/nonzero_px per warp plus the
  Charbonnier flow-smoothness term on device; host sums the 8 per-core
  losses.
"""

import sys

if "/opt/trn_rl_repo" not in sys.path:
    sys.path.insert(0, "/opt/trn_rl_repo")

from contextlib import ExitStack

import ml_dtypes
import numpy as np

import concourse.bacc as bacc
import concourse.bass as bass
import concourse.mybir as mybir
from concourse.tile import TileContext

F32 = mybir.dt.float32
BF16 = mybir.dt.bfloat16
I16 = mybir.dt.int16
I32 = mybir.dt.int32
AL = mybir.AluOpType
ACTF = mybir.ActivationFunctionType

P = 128
RES = 256
NPIX = RES * RES
EPS = 1e-9
FLOW_TEMP_REG = 1e-3
PAD_POS = -1e4  # pad-event coordinate: far out of bounds, zero contribution

NT = 12   # scatter taps per event: 2 y-taps x 2 warps + 4 x-taps x 2 warps
SW = 1536  # scatter tile width: ty0[256] ty1[256] rhs0[512] rhs1[512]
U = 8     # chunks per hw-loop iteration
CB = 80   # chunks per block (dynamic-offset register range: 80*24B < 2KB)


def _emit(tc, ev, vecb, loss_out, C2, mt):
    """C2 = total chunks (pos segment PC + neg segment PC, PC = C2//2)."""
    nc = tc.nc
    PC = C2 // 2
    stk = ExitStack()

    const_pool = stk.enter_context(tc.tile_pool(name="const", bufs=1))
    ones = const_pool.tile([P, 1], F32)
    nc.gpsimd.memset(ones, 1.0)
    zk = const_pool.tile([1, 640], BF16)
    nc.gpsimd.memset(zk, 0.0)
    vtile = const_pool.tile([1, 32], F32)
    nc.sync.dma_start(vtile, vecb)

    # persistent per-event scatter operands: NT taps per event, interleaved
    # per chunk column: idxw/datw[p, NT*c + k]
    pers_pool = stk.enter_context(tc.tile_pool(name="pers", bufs=1))
    idxw = pers_pool.tile([P, NT * C2], I16)
    datw = pers_pool.tile([P, NT * C2], BF16)
    idx4 = idxw[:, 0 : NT * C2].rearrange("p (c f) -> p c f", f=NT)
    dat4 = datw[:, 0 : NT * C2].rearrange("p (c f) -> p c f", f=NT)

    # ---- prep: wide [P, C2] field math on DVE ----
    with tc.tile_pool(name="raw", bufs=1) as raw_pool:
        def load_field(f):
            t = raw_pool.tile([P, C2], F32, tag=f"raw{f}", name=f"raw{f}")
            nc.sync.dma_start(t, ev[f : f + 1, :].rearrange("o (p c) -> (o p) c", p=P))
            return t

        ts_t, y_t, x_t, fy_t, fx_t = [load_field(f) for f in range(5)]

        d0 = raw_pool.tile([P, C2], F32, tag="d0")
        nc.vector.tensor_scalar(d0, ts_t, -1.0, float(mt), AL.mult, AL.add)
        scr = raw_pool.tile([P, C2], F32, tag="scr")
        scr2 = raw_pool.tile([P, C2], F32, tag="scr2")
        scri = raw_pool.tile([P, C2], I32, tag="scri")

        # warped positions: wp[w][axis]; axis 0 = y, 1 = x
        wp = [[raw_pool.tile([P, C2], F32, tag=f"wp{w}{a}", name=f"wp{w}{a}")
               for a in (0, 1)] for w in (0, 1)]
        for w, tw in ((0, d0), (1, ts_t)):
            op = AL.add if w == 0 else AL.subtract
            nc.vector.tensor_tensor(out=scr, in0=tw, in1=fy_t, op=AL.mult)
            nc.vector.tensor_tensor(out=wp[w][0], in0=y_t, in1=scr, op=op)
            nc.vector.tensor_tensor(out=scr, in0=tw, in1=fx_t, op=AL.mult)
            nc.vector.tensor_tensor(out=wp[w][1], in0=x_t, in1=scr, op=op)

        # y_t/fy_t/x_t/fx_t are dead now -- reuse their buffers
        flr = y_t
        f1 = fy_t
        m0 = fx_t
        m1 = x_t

        def tap(dst_k, val):
            """cast f32 idx value into interleaved int16 tap column k"""
            nc.vector.tensor_copy(out=idx4[:, :, dst_k], in_=val)

        def datc(dst_k, val):
            nc.vector.tensor_copy(out=dat4[:, :, dst_k], in_=val)

        # tap layout per event (offsets into the [P,SW] scatter tile):
        #  0: y0   w0 (off 0)      1: y0+1 w0
        #  2: y0   w1 (off 256)    3: y0+1 w1
        #  4: x0   w0 den (512)    5: x0+1 w0 den
        #  6: x0   w0 num (768)    7: x0+1 w0 num
        #  8: x0   w1 den (1024)   9: x0+1 w1 den
        # 10: x0   w1 num (1280)  11: x0+1 w1 num
        for w in (0, 1):
            tw = ts_t if w == 0 else d0
            for a in (0, 1):
                xc = wp[w][a]
                nc.vector.tensor_scalar(xc, xc, 300.0, -4.0, AL.min, AL.max)
                nc.vector.tensor_copy(out=scri, in_=xc)
                nc.vector.tensor_copy(out=flr, in_=scri)
                nc.vector.tensor_tensor(out=scr, in0=flr, in1=xc, op=AL.is_gt)
                nc.vector.tensor_tensor(out=flr, in0=flr, in1=scr, op=AL.subtract)
                nc.vector.tensor_tensor(out=f1, in0=xc, in1=flr, op=AL.subtract)
                # masks: m_k = (flr+k <= 255) & (flr+k >= 0 when offset > 0)
                nc.vector.tensor_scalar(m0, flr, 255.5, None, AL.is_le)
                nc.vector.tensor_scalar(m1, flr, 254.5, None, AL.is_le)
                off0 = 256 * w if a == 0 else None  # y-offsets
                if a == 0 and w == 0:
                    offs = [0]
                elif a == 0:
                    offs = [256]
                else:
                    offs = [512 + 512 * w, 768 + 512 * w]
                need_lower = offs != [0]
                if need_lower:
                    nc.vector.tensor_scalar(scr2, flr, -0.5, None, AL.is_ge)
                    nc.vector.tensor_tensor(out=m0, in0=m0, in1=scr2, op=AL.mult)
                    nc.vector.tensor_scalar(scr2, flr, -1.5, None, AL.is_ge)
                    nc.vector.tensor_tensor(out=m1, in0=m1, in1=scr2, op=AL.mult)
                base_k = (2 * w) if a == 0 else (4 + 4 * w)
                for j, off in enumerate(offs):
                    # i0 = (flr + off + 1)*m0 - 1 ; i1 = (flr + off + 2)*m1 - 1
                    nc.vector.tensor_scalar(scr, flr, float(off + 1), None, AL.add)
                    nc.vector.tensor_tensor(out=scr, in0=scr, in1=m0, op=AL.mult)
                    nc.vector.tensor_scalar(scr, scr, -1.0, None, AL.add)
                    tap(base_k + 2 * j, scr)
                    nc.vector.tensor_scalar(scr, flr, float(off + 2), None, AL.add)
                    nc.vector.tensor_tensor(out=scr, in0=scr, in1=m1, op=AL.mult)
                    nc.vector.tensor_scalar(scr, scr, -1.0, None, AL.add)
                    tap(base_k + 2 * j + 1, scr)
                # data: f0 = 1-f1 at tap0, f1 at tap1 (den);  *tw for num taps
                nc.vector.tensor_scalar(scr, f1, -1.0, 1.0, AL.mult, AL.add)
                datc(base_k, scr)
                datc(base_k + 1, f1)
                if a == 1:
                    nc.vector.tensor_tensor(out=scr, in0=scr, in1=tw, op=AL.mult)
                    datc(base_k + 2, scr)
                    nc.vector.tensor_tensor(out=scr, in0=f1, in1=tw, op=AL.mult)
                    datc(base_k + 3, scr)

    psum_pool = tc.tile_pool(name="psum", bufs=1, space="PSUM")
    psum = psum_pool.__enter__()
    # IMG[pol][w][h]: den in cols 0:256, num in cols 256:512
    IMG = [
        [
            [psum.tile([P, 512], F32, tag=f"I{p_}{w}{h}", name=f"I{p_}{w}{h}")
             for h in (0, 1)]
            for w in (0, 1)
        ]
        for p_ in (0, 1)
    ]

    zl = zk[0:1, 0:128]
    zr = zk[0:1, 128:640]
    for p_ in (0, 1):
        for w in (0, 1):
            for h in (0, 1):
                nc.tensor.matmul(out=IMG[p_][w][h][:], lhsT=zl, rhs=zr,
                                 start=True, stop=False)

    loop_pool = stk.enter_context(tc.tile_pool(name="loop", bufs=3))

    def chunk_body(pol, base, span, i, u):
        iv = idx4[:, base + u : base + span : U, :][:, bass.ds(i, 1), :]
        dv = dat4[:, base + u : base + span : U, :][:, bass.ds(i, 1), :]
        scat = loop_pool.tile([P, SW], BF16, tag=f"s{u}", name=f"s{u}")
        nc.gpsimd.local_scatter(scat[:], dv, iv, channels=P,
                                num_elems=SW, num_idxs=NT)
        for w in (0, 1):
            rhs = scat[:, 512 + 512 * w : 1024 + 512 * w]
            for h in (0, 1):
                nc.tensor.matmul(
                    out=IMG[pol][w][h][:],
                    lhsT=scat[:, 256 * w + 128 * h : 256 * w + 128 * h + 128],
                    rhs=rhs,
                    start=False,
                    stop=False,
                )

    for pol in (0, 1):
        seg0 = pol * PC
        for b in range(0, PC, CB):
            span = min(CB, PC - b)
            with tc.For_i(0, span // U) as i:
                for u in range(U):
                    chunk_body(pol, seg0 + b, span, i, u)

    for p_ in (0, 1):
        for w in (0, 1):
            for h in (0, 1):
                nc.tensor.matmul(out=IMG[p_][w][h][:], lhsT=zl, rhs=zr,
                                 start=False, stop=True)

    # ---- epilogue ----
    epi_pool = stk.enter_context(tc.tile_pool(name="epi", bufs=1))
    rows = epi_pool.tile([P, 4], F32)
    den = epi_pool.tile([P, 256], F32, tag="den")
    num = epi_pool.tile([P, 256], F32, tag="num")
    rec = epi_pool.tile([P, 256], F32, tag="rec")
    for w in (0, 1):
        SQ = epi_pool.tile([P, 256], F32, tag=f"SQ{w}", name=f"SQ{w}")
        Z = epi_pool.tile([P, 256], F32, tag=f"Z{w}", name=f"Z{w}")
        nc.vector.memset(SQ, 0.0)
        nc.vector.memset(Z, 0.0)
        for h in (0, 1):
            Ph, Nh = IMG[0][w][h], IMG[1][w][h]
            for img in (Ph, Nh):
                nc.vector.tensor_scalar(den, img[:, 0:256], EPS, None, AL.add)
                nc.vector.reciprocal(rec, den)
                nc.vector.tensor_tensor(out=num, in0=img[:, 256:512], in1=rec,
                                        op=AL.mult)
                nc.vector.tensor_tensor(out=num, in0=num, in1=num, op=AL.mult)
                nc.vector.tensor_tensor(out=SQ, in0=SQ, in1=num, op=AL.add)
            # nonzero-pixel count uses den_pos + den_neg
            # (only one tensor_tensor input may come from PSUM -> stage N)
            nc.vector.tensor_copy(out=rec, in_=Nh[:, 0:256])
            nc.vector.tensor_tensor(out=den, in0=Ph[:, 0:256], in1=rec, op=AL.add)
            nc.vector.tensor_scalar(den, den, 0.0, None, AL.is_equal)
            nc.vector.tensor_tensor(out=Z, in0=Z, in1=den, op=AL.add)
        nc.vector.tensor_reduce(
            out=rows[:, 2 * w : 2 * w + 1], in_=SQ, axis=mybir.AxisListType.X,
            op=AL.add,
        )
        nc.vector.tensor_reduce(
            out=rows[:, 2 * w + 1 : 2 * w + 2], in_=Z,
            axis=mybir.AxisListType.X, op=AL.add,
        )

    psum_pool.__exit__(None, None, None)

    with tc.tile_pool(name="psum2", bufs=1, space="PSUM") as psum2:
        red = psum2.tile([1, 4], F32)
        nc.tensor.matmul(out=red[:], lhsT=ones[:], rhs=rows[:], start=True,
                         stop=True)
        scal = epi_pool.tile([1, 4], F32)
        nc.vector.tensor_copy(out=scal, in_=red[:])

    lt = epi_pool.tile([1, 1], F32)
    nc.vector.memset(lt, 0.0)
    t1 = epi_pool.tile([1, 1], F32)
    t2 = epi_pool.tile([1, 1], F32)
    for w in (0, 1):
        # t1 = 65536 - zero_count (the reference's +EPS is an f32 no-op here)
        nc.vector.tensor_scalar(
            t1, scal[0:1, 2 * w + 1 : 2 * w + 2], -1.0, float(NPIX), AL.mult,
            AL.add,
        )
        nc.vector.reciprocal(t2, t1)
        nc.vector.tensor_scalar(
            t1, scal[0:1, 2 * w : 2 * w + 1], 1.0 / (mt * mt), None, AL.mult
        )
        nc.vector.scalar_tensor_tensor(lt, t1, t2, lt, AL.mult, AL.add)

    # Charbonnier temporal-smoothness on vector_list
    d24 = epi_pool.tile([1, 24], F32)
    nc.vector.tensor_tensor(
        out=d24, in0=vtile[0:1, 0:24], in1=vtile[0:1, 8:32], op=AL.subtract
    )
    epsb = epi_pool.tile([1, 1], F32)
    nc.vector.memset(epsb, EPS)
    nc.scalar.activation(d24, d24, ACTF.Square)
    nc.scalar.activation(d24, d24, ACTF.Sqrt, bias=epsb[0:1, 0:1])
    ch = epi_pool.tile([1, 1], F32)
    nc.vector.tensor_reduce(out=ch, in_=d24, axis=mybir.AxisListType.X, op=AL.add)
    nc.vector.scalar_tensor_tensor(lt, ch, FLOW_TEMP_REG / 24.0, lt, AL.mult,
                                   AL.add)

    nc.sync.dma_start(loss_out, lt[:])
    stk.close()


def _build(C2, mt, num_devices=8):
    nc = bacc.Bacc(
        "TRN2", target_bir_lowering=False, debug=False, num_devices=num_devices
    )
    N2 = C2 * P
    ev = nc.dram_tensor("ev", [5, N2], F32, kind="ExternalInput")
    vecb = nc.dram_tensor("vecb", [1, 32], F32, kind="ExternalInput")
    loss = nc.dram_tensor("loss", [1, 1], F32, kind="ExternalOutput")
    with TileContext(nc) as tc:
        _emit(tc, ev.ap(), vecb.ap(), loss.ap(), C2, mt)
    nc.compile()
    return nc


def _pack_inputs(event_list, flow, pol_mask):
    """Polarity-partition each batch's events, pad each segment to SEGE
    events, lay out as [5, N2] with field matrices [128, C2] flattened
    row-major (chunk = column)."""
    B, N, _ = event_list.shape
    pos_masks = [pol_mask[b, :, 0] > 0.5 for b in range(B)]
    counts = [int(m.sum()) for m in pos_masks]
    maxseg = max(max(c for c in counts), max(N - c for c in counts))
    SEGE = -(-maxseg // (P * U)) * (P * U)  # multiple of P*U events
    PC = SEGE // P
    C2 = 2 * PC

    maps = []
    for b in range(B):
        m = pos_masks[b]
        fields = np.empty((5, P, C2), np.float32)
        ev5 = np.stack([
            event_list[b, :, 0], event_list[b, :, 1], event_list[b, :, 2],
            flow[b, :, 0], flow[b, :, 1],
        ])  # [5, N] (ts, y, x, fy, fx)
        for seg, sel in ((0, m), (1, ~m)):
            data = ev5[:, sel]  # [5, n]
            n = data.shape[1]
            pad = np.zeros((5, SEGE - n), np.float32)
            pad[1:3, :] = PAD_POS  # y, x out of bounds; ts=0, flow=0
            segdata = np.concatenate([data, pad], axis=1)  # [5, SEGE]
            fields[:, :, seg * PC : (seg + 1) * PC] = segdata.reshape(5, P, PC)
        ev_flat = np.ascontiguousarray(fields.reshape(5, P * C2))
        maps.append({"ev": ev_flat})
    return maps, C2


_NC_CACHE = {}
LAST_RESULT = None  # BassKernelResults of the most recent run (for test.py)


def kernel(event_list, flow, pol_mask, vector_list, max_ts):
    from concourse.bass_utils import run_bass_kernel_spmd

    event_list = np.asarray(event_list)
    flow = np.asarray(flow)
    pol_mask = np.asarray(pol_mask)
    vector_list = np.asarray(vector_list)
    B, N, _ = event_list.shape
    mt = float(np.asarray(max_ts))

    in_maps, C2 = _pack_inputs(event_list, flow, pol_mask)
    for b in range(B):
        in_maps[b]["vecb"] = np.ascontiguousarray(
            vector_list[b].reshape(1, 32), dtype=np.float32
        )

    key = (C2, mt, B)
    nc = _NC_CACHE.get(key)
    if nc is None:
        nc = _build(C2, mt, num_devices=B)
        _NC_CACHE[key] = nc

    res = run_bass_kernel_spmd(nc, in_maps, core_ids=list(range(B)))
    global LAST_RESULT
    LAST_RESULT = res
    vals = np.array(
        [res.results[b]["loss"][0, 0] for b in range(B)], dtype=np.float32
    )
    return np.float32(np.sum(vals, dtype=np.float32))


# revision 26
# speedup vs baseline: 4.7932x; 1.1222x over previous
"""Trainium2 Bass kernel for nn_EventWarping (contrast-maximization event
warping loss).

Strategy (data-parallel over batch, one NeuronCore per batch element):
  Bilinear scatter-add of N=262144 warped events into 256x256 images via the
  TensorEngine outer-product histogram: per chunk of 128 events,
    image[y, (x|x*ts)] += ty[e, y]^T @ rhs[e, (x|x*ts)]
  where ty is the 2-tap y-tent and rhs packs the x-tent and its ts-weighted
  copy.  Events are polarity-sorted on the host (scatter-add is permutation
  invariant), so each chunk touches only its polarity's 4 PSUM banks,
  halving the matmul count vs an unsorted kernel.
  All four per-chunk operand tiles (two y-tents + two 512-wide rhs) are
  built by a single GpSimd local_scatter into one [128,1536] tile from
  precomputed int16 indices and bf16 data (12 taps per event, out-of-bounds
  taps mapped to negative indices which local_scatter drops).  The PE then
  runs 4 (ldweights+matmul) pairs per chunk into 8 PSUM banks.  8 chunks per
  hardware-loop iteration amortize loop scaffolding; DVE and ACT stay out of
  the steady-state loop entirely.
  Epilogue computes sum((num/(den+eps))^2)/mt^2/nonzero_px per warp plus the
  Charbonnier flow-smoothness term on device; host sums the 8 per-core
  losses.
"""

import sys

if "/opt/trn_rl_repo" not in sys.path:
    sys.path.insert(0, "/opt/trn_rl_repo")

from contextlib import ExitStack

import ml_dtypes
import numpy as np

import concourse.bacc as bacc
import concourse.bass as bass
import concourse.mybir as mybir
from concourse.tile import TileContext

F32 = mybir.dt.float32
BF16 = mybir.dt.bfloat16
I16 = mybir.dt.int16
I32 = mybir.dt.int32
AL = mybir.AluOpType
ACTF = mybir.ActivationFunctionType

P = 128
RES = 256
NPIX = RES * RES
EPS = 1e-9
FLOW_TEMP_REG = 1e-3
PAD_POS = -1e4  # pad-event coordinate: far out of bounds, zero contribution

NT = 10   # scatter taps per event: 2 y-taps (warp1) + 4 x-taps x 2 warps
SW = 1280  # scatter tile width: ty1[256] rhs0[512] rhs1[512]
U = 8     # chunks per hw-loop iteration
CB = 96   # chunks per block (dynamic-offset register range: 96*20B < 2KB)


def _emit(tc, ev, iotas, vecb, loss_out, C2, mt):
    """C2 = total chunks (pos segment PC + neg segment PC, PC = C2//2)."""
    nc = tc.nc
    PC = C2 // 2
    stk = ExitStack()

    const_pool = stk.enter_context(tc.tile_pool(name="const", bufs=1))
    iota = const_pool.tile([P, 256], BF16)
    nc.sync.dma_start(iota, iotas[:, 0:256])
    iota_n = const_pool.tile([P, 256], BF16)
    nc.sync.dma_start(iota_n, iotas[:, 256:512])
    ones = const_pool.tile([P, 1], F32)
    nc.gpsimd.memset(ones, 1.0)
    zk = const_pool.tile([1, 640], BF16)
    nc.gpsimd.memset(zk, 0.0)
    vtile = const_pool.tile([1, 32], F32)
    nc.sync.dma_start(vtile, vecb)

    # persistent per-event scatter operands: NT taps per event, interleaved
    # per chunk column: idxw/datw[p, NT*c + k], plus warp0 y-tent ptr scalars
    pers_pool = stk.enter_context(tc.tile_pool(name="pers", bufs=1))
    idxw = pers_pool.tile([P, NT * C2], I16)
    datw = pers_pool.tile([P, NT * C2], BF16)
    wy1m0 = pers_pool.tile([P, C2], F32)  # 1 - wy (warp0)
    wyp10 = pers_pool.tile([P, C2], F32)  # wy + 1 (warp0)
    idx4 = idxw[:, 0 : NT * C2].rearrange("p (c f) -> p c f", f=NT)
    dat4 = datw[:, 0 : NT * C2].rearrange("p (c f) -> p c f", f=NT)

    # ---- prep: wide [P, C2] field math on DVE ----
    with tc.tile_pool(name="raw", bufs=1) as raw_pool:
        def load_field(f):
            t = raw_pool.tile([P, C2], F32, tag=f"raw{f}", name=f"raw{f}")
            nc.sync.dma_start(t, ev[f : f + 1, :].rearrange("o (p c) -> (o p) c", p=P))
            return t

        ts_t, y_t, x_t, fy_t, fx_t = [load_field(f) for f in range(5)]

        d0 = raw_pool.tile([P, C2], F32, tag="d0")
        nc.vector.tensor_scalar(d0, ts_t, -1.0, float(mt), AL.mult, AL.add)
        scr = raw_pool.tile([P, C2], F32, tag="scr")
        scr2 = raw_pool.tile([P, C2], F32, tag="scr2")
        scri = raw_pool.tile([P, C2], I32, tag="scri")

        # warped positions: wp[w][axis]; axis 0 = y, 1 = x
        wp = [[raw_pool.tile([P, C2], F32, tag=f"wp{w}{a}", name=f"wp{w}{a}")
               for a in (0, 1)] for w in (0, 1)]
        for w, tw in ((0, d0), (1, ts_t)):
            op = AL.add if w == 0 else AL.subtract
            nc.vector.tensor_tensor(out=scr, in0=tw, in1=fy_t, op=AL.mult)
            nc.vector.tensor_tensor(out=wp[w][0], in0=y_t, in1=scr, op=op)
            nc.vector.tensor_tensor(out=scr, in0=tw, in1=fx_t, op=AL.mult)
            nc.vector.tensor_tensor(out=wp[w][1], in0=x_t, in1=scr, op=op)

        # y_t/fy_t/x_t/fx_t are dead now -- reuse their buffers
        flr = y_t
        f1 = fy_t
        m0 = fx_t
        m1 = x_t

        def tap(dst_k, val):
            """cast f32 idx value into interleaved int16 tap column k"""
            nc.vector.tensor_copy(out=idx4[:, :, dst_k], in_=val)

        def datc(dst_k, val):
            nc.vector.tensor_copy(out=dat4[:, :, dst_k], in_=val)

        # warp0 y-tent uses the DVE+ACT path: precompute its ptr scalars
        nc.vector.tensor_scalar(wy1m0, wp[0][0], -1.0, 1.0, AL.mult, AL.add)
        nc.vector.tensor_scalar(wyp10, wp[0][0], 1.0, None, AL.add)

        # tap layout per event (offsets into the [P,SW] scatter tile):
        #  0: y0   w1 (off 0)      1: y0+1 w1
        #  2: x0   w0 den (256)    3: x0+1 w0 den
        #  4: x0   w0 num (512)    5: x0+1 w0 num
        #  6: x0   w1 den (768)    7: x0+1 w1 den
        #  8: x0   w1 num (1024)   9: x0+1 w1 num
        for w in (0, 1):
            tw = ts_t if w == 0 else d0
            for a in (0, 1):
                if a == 0 and w == 0:
                    continue  # warp0 y-tent built on DVE/ACT in the loop
                xc = wp[w][a]
                nc.vector.tensor_scalar(xc, xc, 300.0, -4.0, AL.min, AL.max)
                nc.vector.tensor_copy(out=scri, in_=xc)
                nc.vector.tensor_copy(out=flr, in_=scri)
                nc.vector.tensor_tensor(out=scr, in0=flr, in1=xc, op=AL.is_gt)
                nc.vector.tensor_tensor(out=flr, in0=flr, in1=scr, op=AL.subtract)
                nc.vector.tensor_tensor(out=f1, in0=xc, in1=flr, op=AL.subtract)
                # masks: m_k = (flr+k <= 255) & (flr+k >= 0 when offset > 0)
                nc.vector.tensor_scalar(m0, flr, 255.5, None, AL.is_le)
                nc.vector.tensor_scalar(m1, flr, 254.5, None, AL.is_le)
                if a == 0:
                    offs = [0]
                else:
                    offs = [256 + 512 * w, 512 + 512 * w]
                need_lower = offs != [0]
                if need_lower:
                    nc.vector.tensor_scalar(scr2, flr, -0.5, None, AL.is_ge)
                    nc.vector.tensor_tensor(out=m0, in0=m0, in1=scr2, op=AL.mult)
                    nc.vector.tensor_scalar(scr2, flr, -1.5, None, AL.is_ge)
                    nc.vector.tensor_tensor(out=m1, in0=m1, in1=scr2, op=AL.mult)
                base_k = 0 if a == 0 else (2 + 4 * w)
                for j, off in enumerate(offs):
                    # i0 = (flr + off + 1)*m0 - 1 ; i1 = (flr + off + 2)*m1 - 1
                    nc.vector.tensor_scalar(scr, flr, float(off + 1), None, AL.add)
                    nc.vector.tensor_tensor(out=scr, in0=scr, in1=m0, op=AL.mult)
                    nc.vector.tensor_scalar(scr, scr, -1.0, None, AL.add)
                    tap(base_k + 2 * j, scr)
                    nc.vector.tensor_scalar(scr, flr, float(off + 2), None, AL.add)
                    nc.vector.tensor_tensor(out=scr, in0=scr, in1=m1, op=AL.mult)
                    nc.vector.tensor_scalar(scr, scr, -1.0, None, AL.add)
                    tap(base_k + 2 * j + 1, scr)
                # data: f0 = 1-f1 at tap0, f1 at tap1 (den);  *tw for num taps
                nc.vector.tensor_scalar(scr, f1, -1.0, 1.0, AL.mult, AL.add)
                datc(base_k, scr)
                datc(base_k + 1, f1)
                if a == 1:
                    nc.vector.tensor_tensor(out=scr, in0=scr, in1=tw, op=AL.mult)
                    datc(base_k + 2, scr)
                    nc.vector.tensor_tensor(out=scr, in0=f1, in1=tw, op=AL.mult)
                    datc(base_k + 3, scr)

    psum_pool = tc.tile_pool(name="psum", bufs=1, space="PSUM")
    psum = psum_pool.__enter__()
    # IMG[pol][w][h]: den in cols 0:256, num in cols 256:512
    IMG = [
        [
            [psum.tile([P, 512], F32, tag=f"I{p_}{w}{h}", name=f"I{p_}{w}{h}")
             for h in (0, 1)]
            for w in (0, 1)
        ]
        for p_ in (0, 1)
    ]

    zl = zk[0:1, 0:128]
    zr = zk[0:1, 128:640]
    for p_ in (0, 1):
        for w in (0, 1):
            for h in (0, 1):
                nc.tensor.matmul(out=IMG[p_][w][h][:], lhsT=zl, rhs=zr,
                                 start=True, stop=False)

    loop_pool = stk.enter_context(tc.tile_pool(name="loop", bufs=3))

    def chunk_body(pol, base, span, i, u):
        iv = idx4[:, base + u : base + span : U, :][:, bass.ds(i, 1), :]
        dv = dat4[:, base + u : base + span : U, :][:, bass.ds(i, 1), :]
        scat = loop_pool.tile([P, SW], BF16, tag=f"s{u}", name=f"s{u}")
        nc.gpsimd.local_scatter(scat[:], dv, iv, channels=P,
                                num_elems=SW, num_idxs=NT)
        # warp0 y-tent: ut = 1+d, tp = min(1-d, 1+d), ty0 = relu(tp)
        w1c = wy1m0[:, base + u : base + span : U][:, bass.ds(i, 1)]
        p1c = wyp10[:, base + u : base + span : U][:, bass.ds(i, 1)]
        ut = loop_pool.tile([P, 256], BF16, tag=f"u{u}", name=f"u{u}")
        nc.vector.tensor_scalar(ut, iota, w1c, None, AL.add)
        tp = loop_pool.tile([P, 256], BF16, tag=f"p{u}", name=f"p{u}")
        nc.vector.scalar_tensor_tensor(tp, iota_n, p1c, ut, AL.add, AL.min)
        ty0 = loop_pool.tile([P, 256], BF16, tag=f"t{u}", name=f"t{u}")
        nc.scalar.activation(ty0, tp, ACTF.Relu)
        for w in (0, 1):
            rhs = scat[:, 256 + 512 * w : 768 + 512 * w]
            for h in (0, 1):
                lhsT = (ty0[:, 128 * h : 128 * h + 128] if w == 0
                        else scat[:, 128 * h : 128 * h + 128])
                nc.tensor.matmul(
                    out=IMG[pol][w][h][:],
                    lhsT=lhsT,
                    rhs=rhs,
                    start=False,
                    stop=False,
                )

    for pol in (0, 1):
        seg0 = pol * PC
        for b in range(0, PC, CB):
            span = min(CB, PC - b)
            with tc.For_i(0, span // U) as i:
                for u in range(U):
                    chunk_body(pol, seg0 + b, span, i, u)

    for p_ in (0, 1):
        for w in (0, 1):
            for h in (0, 1):
                nc.tensor.matmul(out=IMG[p_][w][h][:], lhsT=zl, rhs=zr,
                                 start=False, stop=True)

    # ---- epilogue ----
    epi_pool = stk.enter_context(tc.tile_pool(name="epi", bufs=1))
    rows = epi_pool.tile([P, 4], F32)
    den = epi_pool.tile([P, 256], F32, tag="den")
    num = epi_pool.tile([P, 256], F32, tag="num")
    rec = epi_pool.tile([P, 256], F32, tag="rec")
    for w in (0, 1):
        SQ = epi_pool.tile([P, 256], F32, tag=f"SQ{w}", name=f"SQ{w}")
        Z = epi_pool.tile([P, 256], F32, tag=f"Z{w}", name=f"Z{w}")
        nc.vector.memset(SQ, 0.0)
        nc.vector.memset(Z, 0.0)
        for h in (0, 1):
            Ph, Nh = IMG[0][w][h], IMG[1][w][h]
            for img in (Ph, Nh):
                nc.vector.tensor_scalar(den, img[:, 0:256], EPS, None, AL.add)
                nc.vector.reciprocal(rec, den)
                nc.vector.tensor_tensor(out=num, in0=img[:, 256:512], in1=rec,
                                        op=AL.mult)
                nc.vector.tensor_tensor(out=num, in0=num, in1=num, op=AL.mult)
                nc.vector.tensor_tensor(out=SQ, in0=SQ, in1=num, op=AL.add)
            # nonzero-pixel count uses den_pos + den_neg
            # (only one tensor_tensor input may come from PSUM -> stage N)
            nc.vector.tensor_copy(out=rec, in_=Nh[:, 0:256])
            nc.vector.tensor_tensor(out=den, in0=Ph[:, 0:256], in1=rec, op=AL.add)
            nc.vector.tensor_scalar(den, den, 0.0, None, AL.is_equal)
            nc.vector.tensor_tensor(out=Z, in0=Z, in1=den, op=AL.add)
        nc.vector.tensor_reduce(
            out=rows[:, 2 * w : 2 * w + 1], in_=SQ, axis=mybir.AxisListType.X,
            op=AL.add,
        )
        nc.vector.tensor_reduce(
            out=rows[:, 2 * w + 1 : 2 * w + 2], in_=Z,
            axis=mybir.AxisListType.X, op=AL.add,
        )

    psum_pool.__exit__(None, None, None)

    with tc.tile_pool(name="psum2", bufs=1, space="PSUM") as psum2:
        red = psum2.tile([1, 4], F32)
        nc.tensor.matmul(out=red[:], lhsT=ones[:], rhs=rows[:], start=True,
                         stop=True)
        scal = epi_pool.tile([1, 4], F32)
        nc.vector.tensor_copy(out=scal, in_=red[:])

    lt = epi_pool.tile([1, 1], F32)
    nc.vector.memset(lt, 0.0)
    t1 = epi_pool.tile([1, 1], F32)
    t2 = epi_pool.tile([1, 1], F32)
    for w in (0, 1):
        # t1 = 65536 - zero_count (the reference's +EPS is an f32 no-op here)
        nc.vector.tensor_scalar(
            t1, scal[0:1, 2 * w + 1 : 2 * w + 2], -1.0, float(NPIX), AL.mult,
            AL.add,
        )
        nc.vector.reciprocal(t2, t1)
        nc.vector.tensor_scalar(
            t1, scal[0:1, 2 * w : 2 * w + 1], 1.0 / (mt * mt), None, AL.mult
        )
        nc.vector.scalar_tensor_tensor(lt, t1, t2, lt, AL.mult, AL.add)

    # Charbonnier temporal-smoothness on vector_list
    d24 = epi_pool.tile([1, 24], F32)
    nc.vector.tensor_tensor(
        out=d24, in0=vtile[0:1, 0:24], in1=vtile[0:1, 8:32], op=AL.subtract
    )
    epsb = epi_pool.tile([1, 1], F32)
    nc.vector.memset(epsb, EPS)
    nc.scalar.activation(d24, d24, ACTF.Square)
    nc.scalar.activation(d24, d24, ACTF.Sqrt, bias=epsb[0:1, 0:1])
    ch = epi_pool.tile([1, 1], F32)
    nc.vector.tensor_reduce(out=ch, in_=d24, axis=mybir.AxisListType.X, op=AL.add)
    nc.vector.scalar_tensor_tensor(lt, ch, FLOW_TEMP_REG / 24.0, lt, AL.mult,
                                   AL.add)

    nc.sync.dma_start(loss_out, lt[:])
    stk.close()


def _build(C2, mt, num_devices=8):
    nc = bacc.Bacc(
        "TRN2", target_bir_lowering=False, debug=False, num_devices=num_devices
    )
    N2 = C2 * P
    ev = nc.dram_tensor("ev", [5, N2], F32, kind="ExternalInput")
    iotas = nc.dram_tensor("iotas", [P, 512], BF16, kind="ExternalInput")
    vecb = nc.dram_tensor("vecb", [1, 32], F32, kind="ExternalInput")
    loss = nc.dram_tensor("loss", [1, 1], F32, kind="ExternalOutput")
    with TileContext(nc) as tc:
        _emit(tc, ev.ap(), iotas.ap(), vecb.ap(), loss.ap(), C2, mt)
    nc.compile()
    return nc


def _host_iotas():
    a = np.arange(256, dtype=np.float32)
    io = np.concatenate([a, -a])
    return np.tile(io[None, :], (P, 1)).astype(ml_dtypes.bfloat16)


def _pack_inputs(event_list, flow, pol_mask):
    """Polarity-partition each batch's events, pad each segment to SEGE
    events, lay out as [5, N2] with field matrices [128, C2] flattened
    row-major (chunk = column)."""
    B, N, _ = event_list.shape
    pos_masks = [pol_mask[b, :, 0] > 0.5 for b in range(B)]
    counts = [int(m.sum()) for m in pos_masks]
    maxseg = max(max(c for c in counts), max(N - c for c in counts))
    SEGE = -(-maxseg // (P * U)) * (P * U)  # multiple of P*U events
    PC = SEGE // P
    C2 = 2 * PC

    iot = _host_iotas()
    maps = []
    for b in range(B):
        m = pos_masks[b]
        fields = np.empty((5, P, C2), np.float32)
        ev5 = np.stack([
            event_list[b, :, 0], event_list[b, :, 1], event_list[b, :, 2],
            flow[b, :, 0], flow[b, :, 1],
        ])  # [5, N] (ts, y, x, fy, fx)
        for seg, sel in ((0, m), (1, ~m)):
            data = ev5[:, sel]  # [5, n]
            n = data.shape[1]
            pad = np.zeros((5, SEGE - n), np.float32)
            pad[1:3, :] = PAD_POS  # y, x out of bounds; ts=0, flow=0
            segdata = np.concatenate([data, pad], axis=1)  # [5, SEGE]
            fields[:, :, seg * PC : (seg + 1) * PC] = segdata.reshape(5, P, PC)
        ev_flat = np.ascontiguousarray(fields.reshape(5, P * C2))
        maps.append({"ev": ev_flat, "iotas": iot})
    return maps, C2


_NC_CACHE = {}
LAST_RESULT = None  # BassKernelResults of the most recent run (for test.py)


def kernel(event_list, flow, pol_mask, vector_list, max_ts):
    from concourse.bass_utils import run_bass_kernel_spmd

    event_list = np.asarray(event_list)
    flow = np.asarray(flow)
    pol_mask = np.asarray(pol_mask)
    vector_list = np.asarray(vector_list)
    B, N, _ = event_list.shape
    mt = float(np.asarray(max_ts))

    in_maps, C2 = _pack_inputs(event_list, flow, pol_mask)
    for b in range(B):
        in_maps[b]["vecb"] = np.ascontiguousarray(
            vector_list[b].reshape(1, 32), dtype=np.float32
        )

    key = (C2, mt, B)
    nc = _NC_CACHE.get(key)
    if nc is None:
        nc = _build(C2, mt, num_devices=B)
        _NC_CACHE[key] = nc

    res = run_bass_kernel_spmd(nc, in_maps, core_ids=list(range(B)))
    global LAST_RESULT
    LAST_RESULT = res
    vals = np.array(
        [res.results[b]["loss"][0, 0] for b in range(B)], dtype=np.float32
    )
    return np.float32(np.sum(vals, dtype=np.float32))
